# revision 17
# baseline (speedup 1.0000x reference)
"""Multi-head attention block (B=16, N=577, C=1024, H=16) on 8 Trainium2 NeuronCores.

Sharding: data-parallel over batch — 2 batch elements per core, no collectives.

Device dataflow per batch element (fully "transposed" so no on-device transposes):
  inputs staged host-side: xT = x^T  [C,N] bf16, wqkvT = w_qkv^T [C,3C] bf16,
  wprojT = w_proj^T [C,C] bf16.
  qT,kT [o,n] <- (wqkvT tile).T @ xT      (o on partitions: per-head [64, n])
  V     [n,o] <- (xT tile).T @ wqkvT      (n on partitions: per-head [m, 64])
  S^T   [m,n] <- (kT_h tile [d,m]).T @ qT_h [d,n]          (d=64 contraction)
  P^T = exp(0.125 * S^T)                  (softmax numerator; max-subtraction skipped:
                                           scaled scores are ~N(0,1), |s|<~10, exp safe)
  OT'[0:64,n] = sum_m V_h[m,d] P^T[m,n];  OT'[64,n] = sum_m P^T[m,n]
      (one matmul: lhsT = [V_h | ones] [m, 65] — sumexp comes free as row 64)
  OT = OT'[0:64] * (1/OT'[64])            (softmax denominator)
  y[n,o] = (OT tile [c,n]).T @ wprojT + b_proj
"""

import os
import sys

import numpy as np

if "/opt/trn_rl_repo" not in sys.path:
    sys.path.insert(0, "/opt/trn_rl_repo")

import ml_dtypes

B, N, C = 16, 577, 1024
H, D = 16, 64
P = 128
CT = C // P  # 8 contraction tiles
NT = 5  # n(row) tiles of 128: 4*128 + 65
NTS = [128, 128, 128, 128, 65]
NCH = [(0, 512), (512, 65)]  # free-dim chunks of 577 (psum bank = 512 fp32)
NCORES = 8
BPC = B // NCORES  # batches per core

_CACHE = {}
LAST_RESULT = None


def _ensure_ntff_hook():
    try:
        from antenv import axon_hooks  # noqa: F401

        return
    except ImportError:
        pass
    import contextlib
    import ctypes
    import types

    import antenv

    so_path = "/opt/axon/libaxon_pjrt.so"
    mod = types.ModuleType("antenv.axon_hooks")
    _state = {"hook": None, "set": False}

    def _make_hook():
        if not os.path.exists(so_path):
            return None
        lib = ctypes.CDLL(so_path)
        if not hasattr(lib, "axon_start_nrt_profile"):
            return None
        lib.axon_start_nrt_profile.argtypes = [
            ctypes.POINTER(ctypes.c_int64),
            ctypes.c_size_t,
        ]
        lib.axon_start_nrt_profile.restype = ctypes.c_int64
        lib.axon_stop_nrt_profile.argtypes = [ctypes.c_char_p]
        lib.axon_stop_nrt_profile.restype = ctypes.c_int64

        @contextlib.contextmanager
        def _hook(output_dir, device_ids):
            import jax

            jax.devices()
            if device_ids:
                ids = (ctypes.c_int64 * len(device_ids))(*device_ids)
                rc = lib.axon_start_nrt_profile(ids, len(device_ids))
            else:
                rc = lib.axon_start_nrt_profile(None, 0)
            if rc != 0:
                raise RuntimeError(f"axon_start_nrt_profile rc={rc}")
            try:
                yield
            finally:
                n = lib.axon_stop_nrt_profile(str(output_dir).encode())
                print(f"ntff profile: {n} file(s) written to {output_dir}", file=sys.stderr)

        return _hook

    def set_axon_ntff_profile_hook(h):
        _state["hook"] = h
        _state["set"] = True

    def get_axon_ntff_profile_hook():
        if not _state["set"]:
            set_axon_ntff_profile_hook(_make_hook())
        return _state["hook"]

    mod.set_axon_ntff_profile_hook = set_axon_ntff_profile_hook
    mod.get_axon_ntff_profile_hook = get_axon_ntff_profile_hook
    sys.modules["antenv.axon_hooks"] = mod
    antenv.axon_hooks = mod


def _build_nc():
    import concourse.bass as bass
    import concourse.tile as tile
    from concourse import bacc, mybir

    dtb = mybir.dt.bfloat16
    dtf = mybir.dt.float32
    Exp = mybir.ActivationFunctionType.Exp

    nc = bacc.Bacc(None, target_bir_lowering=False)

    xt = nc.dram_tensor("xt", [BPC, C, N], dtb, kind="ExternalInput")
    wq = nc.dram_tensor("wqkvT", [C, 3 * C], dtb, kind="ExternalInput")
    wp = nc.dram_tensor("wprojT", [C, C], dtb, kind="ExternalInput")
    bqk = nc.dram_tensor("bqk", [P, 16], dtf, kind="ExternalInput")
    bv = nc.dram_tensor("bv", [C], dtb, kind="ExternalInput")
    bpr = nc.dram_tensor("bproj", [C], dtb, kind="ExternalInput")
    idn = nc.dram_tensor("idn", [P, P], dtb, kind="ExternalInput")
    y = nc.dram_tensor("y", [BPC, N, C], dtb, kind="ExternalOutput")

    from contextlib import ExitStack

    with tile.TileContext(nc) as tc:
        with ExitStack() as ctx:
            consts = ctx.enter_context(tc.tile_pool(name="consts", bufs=1))
            wpool = ctx.enter_context(tc.tile_pool(name="weights", bufs=1))
            xpool = ctx.enter_context(tc.tile_pool(name="xin", bufs=2))
            qkpool = ctx.enter_context(tc.tile_pool(name="qk", bufs=2))
            vpool = ctx.enter_context(tc.tile_pool(name="vv", bufs=2))
            epool = ctx.enter_context(tc.tile_pool(name="est", bufs=4))
            opool = ctx.enter_context(tc.tile_pool(name="ot", bufs=2))
            rpool = ctx.enter_context(tc.tile_pool(name="rp", bufs=4))
            onpool = ctx.enter_context(tc.tile_pool(name="on", bufs=2))
            outpool = ctx.enter_context(tc.tile_pool(name="outs", bufs=2))
            psA = ctx.enter_context(tc.tile_pool(name="psA", bufs=3, space="PSUM"))
            psB = ctx.enter_context(tc.tile_pool(name="psB", bufs=2, space="PSUM"))

            wq_sb = wpool.tile([P, CT, 3 * C], dtb, tag="wq")
            wp_sb = wpool.tile([P, CT, C], dtb, tag="wp")
            bqk_sb = consts.tile([P, 16], dtf, tag="bqk")
            ones1 = consts.tile([1, P], dtb, tag="ones1")
            nc.vector.memset(ones1[:], 1.0)
            wdum = consts.tile([1, 512], dtb, tag="wdum")
            nc.vector.memset(wdum[:], 1.0)
            pwarm = psB.tile([P, 512], dtf, tag="psB", name="pwarm")
            for i in range(9):
                nc.tensor.matmul(
                    pwarm[:], lhsT=ones1[0:1, :], rhs=wdum[0:1, :],
                    start=(i == 0), stop=(i == 8),
                )
            idn_sb = consts.tile([P, P], dtb, tag="idn")
            bvb_sb = consts.tile([P, C], dtb, tag="bvb")
            bpb_sb = consts.tile([P, C], dtb, tag="bpb")

            def load_x(b):
                x_sb = xpool.tile([P, CT, N], dtb, tag="x")
                xb = xt[b].rearrange("(ct p) n -> p ct n", p=P)
                for ct in range(CT):
                    eng = nc.sync if ct % 2 == 0 else nc.gpsimd
                    eng.dma_start(out=x_sb[:, ct], in_=xb[:, ct])
                return x_sb

            def emit_qk_tile(x_sb, qk_sb, ot, wide):
                if wide:
                    ps = psA.tile([P, 640], dtf, tag="psA")
                    for (c0, cw) in NCH:
                        for ct in range(CT):
                            nc.tensor.matmul(
                                ps[:, c0:c0 + cw],
                                lhsT=wq_sb[:, ct, ot * P:(ot + 1) * P],
                                rhs=x_sb[:, ct, c0:c0 + cw],
                                start=(ct == 0),
                                stop=(ct == CT - 1),
                            )
                    nc.vector.tensor_scalar_add(
                        out=qk_sb[:, ot, :],
                        in0=ps[:, :N],
                        scalar1=bqk_sb[:, ot:ot + 1],
                    )
                    return
                for (c0, cw) in NCH:
                    ps = psB.tile([P, 512], dtf, tag="psB")
                    for ct in range(CT):
                        nc.tensor.matmul(
                            ps[:, :cw],
                            lhsT=wq_sb[:, ct, ot * P:(ot + 1) * P],
                            rhs=x_sb[:, ct, c0:c0 + cw],
                            start=(ct == 0),
                            stop=(ct == CT - 1),
                        )
                    nc.vector.tensor_scalar_add(
                        out=qk_sb[:, ot, c0:c0 + cw],
                        in0=ps[:, :cw],
                        scalar1=bqk_sb[:, ot:ot + 1],
                    )

            def emit_v_chunk(x_sb, v_sb, nt, oc):
                nh = NTS[nt]
                ps = psB.tile([P, 512], dtf, tag="psB")
                for ct in range(CT):
                    nc.tensor.matmul(
                        ps[:nh],
                        lhsT=x_sb[:, ct, nt * P:nt * P + nh],
                        rhs=wq_sb[:, ct, 2 * C + oc * 512:2 * C + (oc + 1) * 512],
                        start=(ct == 0),
                        stop=(ct == CT - 1),
                    )
                vv = v_sb[:nh, nt, oc * 8 * 65:(oc + 1) * 8 * 65].rearrange(
                    "p (h c) -> p h c", c=65
                )
                nc.vector.tensor_add(
                    out=vv[:, :, 0:64],
                    in0=ps[:nh].rearrange("p (h c) -> p h c", c=64),
                    in1=bvb_sb[:nh, oc * 512:(oc + 1) * 512].rearrange(
                        "p (h c) -> p h c", c=64
                    ),
                )

            # ---- filler quanta: FIFO-pumped inside pairs so the PE keeps
            # streaming while ACT drains exp PSUM tiles. Strict FIFO keeps at
            # most one unit "open", so psB slot lifetimes never interleave.
            from collections import deque
            fq = deque()

            def pump(n=1):
                for _ in range(n):
                    if fq:
                        fq.popleft()()

            def qk_fill_quanta(x_sb, qk_sb, ot):
                box = {}

                def qa(ot=ot):
                    box["ps"] = psB.tile([P, 512], dtf, tag="psB", name="qkf")
                    for ct in range(4):
                        nc.tensor.matmul(
                            box["ps"][:, :512],
                            lhsT=wq_sb[:, ct, ot * P:(ot + 1) * P],
                            rhs=x_sb[:, ct, 0:512],
                            start=(ct == 0),
                            stop=False,
                        )

                def qb(ot=ot):
                    ps = box["ps"]
                    for ct in range(4, 8):
                        nc.tensor.matmul(
                            ps[:, :512],
                            lhsT=wq_sb[:, ct, ot * P:(ot + 1) * P],
                            rhs=x_sb[:, ct, 0:512],
                            start=False,
                            stop=(ct == 7),
                        )
                    nc.vector.tensor_scalar_add(
                        out=qk_sb[:, ot, 0:512],
                        in0=ps[:, :512],
                        scalar1=bqk_sb[:, ot:ot + 1],
                    )

                def qc(ot=ot):
                    ps2 = psB.tile([P, 512], dtf, tag="psB", name="qkf2")
                    for ct in range(CT):
                        nc.tensor.matmul(
                            ps2[:, :65],
                            lhsT=wq_sb[:, ct, ot * P:(ot + 1) * P],
                            rhs=x_sb[:, ct, 512:577],
                            start=(ct == 0),
                            stop=(ct == CT - 1),
                        )
                    nc.vector.tensor_scalar_add(
                        out=qk_sb[:, ot, 512:577],
                        in0=ps2[:, :65],
                        scalar1=bqk_sb[:, ot:ot + 1],
                    )

                return [qa, qb, qc]

            def v_fill_quanta(x_sb, v_sb, nt, oc):
                box = {}
                nh = NTS[nt]

                def seg(s):
                    ps = box["ps"]
                    for ct in range(s * 4, s * 4 + 4):
                        nc.tensor.matmul(
                            ps[:nh],
                            lhsT=x_sb[:, ct, nt * P:nt * P + nh],
                            rhs=wq_sb[:, ct,
                                      2 * C + oc * 512:2 * C + (oc + 1) * 512],
                            start=(ct == 0),
                            stop=(ct == CT - 1),
                        )

                def va():
                    box["ps"] = psB.tile([P, 512], dtf, tag="psB", name="vf")
                    seg(0)

                def vb():
                    seg(1)
                    vv = v_sb[:nh, nt, oc * 8 * 65:(oc + 1) * 8 * 65].rearrange(
                        "p (h c) -> p h c", c=65
                    )
                    nc.vector.tensor_add(
                        out=vv[:, :, 0:64],
                        in0=box["ps"][:nh].rearrange("p (h c) -> p h c", c=64),
                        in1=bvb_sb[:nh, oc * 512:(oc + 1) * 512].rearrange(
                            "p (h c) -> p h c", c=64
                        ),
                    )

                return [va, vb]

            def proj_fill_quanta(ot_sb, b, nt, oc):
                box = {}

                def pa():
                    box["ps"] = psB.tile([P, 512], dtf, tag="psB", name="pjf")
                    emit_proj_seg(ot_sb, b, nt, oc, box["ps"], 0)

                def pb():
                    emit_proj_seg(ot_sb, b, nt, oc, box["ps"], 1)

                return [pa, pb]

            def emit_transpose(otn_sb, ott_sb, ct):
                """OT [n, c-slice of pair ct] -> OT^T [c-slice, n] via 5 PE
                transposes into one bf16 psum bank, single ACT evac."""
                pst = psB.tile([P, 640], dtb, tag="psB", name="tp")
                for nb in range(NT):
                    nh2 = NTS[nb]
                    nc.tensor.transpose(
                        out=pst[:, nb * P:nb * P + nh2],
                        in_=otn_sb[:nh2, nb, ct * P:(ct + 1) * P],
                        identity=idn_sb[:nh2, :nh2],
                    )
                nc.scalar.copy(out=ott_sb[:, ct, :], in_=pst[:, :N])

            def alloc_v(b):
                v_sb = vpool.tile([P, NT, H * 65], dtb, tag="v")
                v4 = v_sb[:].rearrange("p nt (h c) -> p nt h c", c=65)
                nc.vector.memset(v4[:, :, :, 64], 1.0)
                return v_sb

            def emit_pair(qk_sb, v_sb, ot_sb, hp, st_pumps=(1, 3)):
                ob = hp
                estA = epool.tile([P, NT, N], dtb, tag="est")
                estB = epool.tile([P, NT, N], dtb, tag="est")
                for mt in range(NT):
                    mh = NTS[mt]
                    psa = psA.tile([P, 640], dtf, tag="psA")
                    psb = psA.tile([P, 640], dtf, tag="psA")
                    for (c0, cw) in NCH:
                        nc.tensor.matmul(
                            psa[:mh, c0:c0 + cw],
                            lhsT=qk_sb[0:64, 8 + ob, mt * P:mt * P + mh],
                            rhs=qk_sb[0:64, ob, c0:c0 + cw],
                        )
                        nc.tensor.matmul(
                            psb[:mh, c0:c0 + cw],
                            lhsT=qk_sb[64:128, 8 + ob, mt * P:mt * P + mh],
                            rhs=qk_sb[64:128, ob, c0:c0 + cw],
                        )
                    nc.scalar.activation(
                        out=estA[:mh, mt, :], in_=psa[:mh, :N], func=Exp, scale=0.125
                    )
                    nc.scalar.activation(
                        out=estB[:mh, mt, :], in_=psb[:mh, :N], func=Exp, scale=0.125
                    )
                    if mt in st_pumps:
                        pump(1)
                # PV per head: one 2-bank psA tile holds all 5 n-block
                # chunks at 65-col offsets (bank 0), so the PE streams 25
                # back-to-back 65-col matmuls per head with no evac gating.
                recs = rpool.tile([P, 10], dtf, tag="rec")
                for hi, est in ((0, estA), (1, estB)):
                    h = 2 * hp + hi
                    pvt = psA.tile([P, 640], dtf, tag="psA", name="pv")
                    for nb in range(NT):
                        nh2 = NTS[nb]
                        for mt in range(NT):
                            mh = NTS[mt]
                            nc.tensor.matmul(
                                pvt[:nh2, nb * 65:nb * 65 + 65],
                                lhsT=est[:mh, mt, nb * P:nb * P + nh2],
                                rhs=v_sb[:mh, mt, h * 65:h * 65 + 65],
                                start=(mt == 0),
                                stop=(mt == NT - 1),
                            )
                    pv = pvt[:, :]
                    rc = recs[:, :]
                    on = ot_sb[:, :, :]
                    pstr = pv.ap[0][0]
                    # batched softmax denominators: the 5 sumexp columns
                    # (stride 65) in one reciprocal
                    nc.vector.reciprocal_approx_fast(
                        out=recs[:, hi * 5:hi * 5 + 5],
                        in_=bass.AP(tensor=pv.tensor, offset=pv.offset + 64,
                                    ap=[[pstr, P], [65, NT]]),
                    )
                    # normalize+evac: nb0-3 in one 0-stride-broadcast mul
                    nc.vector.tensor_mul(
                        out=bass.AP(tensor=on.tensor,
                                    offset=on.offset + h * 64,
                                    ap=[[on.ap[0][0], P], [H * 64, 4], [1, 64]]),
                        in0=bass.AP(tensor=pv.tensor, offset=pv.offset,
                                    ap=[[pstr, P], [65, 4], [1, 64]]),
                        in1=bass.AP(tensor=rc.tensor,
                                    offset=rc.offset + hi * 5,
                                    ap=[[rc.ap[0][0], P], [1, 4], [0, 64]]),
                    )
                    nc.vector.tensor_mul(
                        out=bass.AP(tensor=on.tensor,
                                    offset=on.offset + 4 * H * 64 + h * 64,
                                    ap=[[on.ap[0][0], 65], [1, 64]]),
                        in0=bass.AP(tensor=pv.tensor, offset=pv.offset + 260,
                                    ap=[[pstr, 65], [1, 64]]),
                        in1=bass.AP(tensor=rc.tensor,
                                    offset=rc.offset + hi * 5 + 4,
                                    ap=[[rc.ap[0][0], 65], [0, 64]]),
                    )
                    if hi == 0:
                        pump(1)

            def emit_proj_seg(ot_sb, b, nt, oc, ps, seg, evac_act=False):
                nh = NTS[nt]
                for ct in range(seg * 4, seg * 4 + 4):
                    nc.tensor.matmul(
                        ps[:nh],
                        lhsT=ot_sb[:, ct, nt * P:nt * P + nh],
                        rhs=wp_sb[:, ct, oc * 512:(oc + 1) * 512],
                        start=(ct == 0),
                        stop=(ct == CT - 1 and not evac_act),
                    )
                if seg == 1:
                    outt = outpool.tile([P, 512], dtb, tag="out")
                    if evac_act:
                        nc.tensor.matmul(
                            ps[:nh],
                            lhsT=ones1[0:1, :nh],
                            rhs=bpb_sb[0:1, oc * 512:(oc + 1) * 512],
                            start=False,
                            stop=True,
                        )
                        nc.scalar.copy(out=outt[:nh], in_=ps[:nh])
                    else:
                        nc.vector.tensor_add(
                            out=outt[:nh],
                            in0=ps[:nh],
                            in1=bpb_sb[:nh, oc * 512:(oc + 1) * 512],
                        )
                    if b == 1:
                        engs = [nc.sync, nc.gpsimd, nc.scalar]
                        eng = engs[(2 * nt + oc) % 3]
                    else:
                        eng = nc.sync if (nt + oc) % 2 == 0 else nc.gpsimd
                    eng.dma_start(
                        out=y[b, nt * P:nt * P + nh, oc * 512:(oc + 1) * 512],
                        in_=outt[:nh],
                    )

            def emit_proj_chunk(ot_sb, b, nt, oc, wide=False, evac_act=False):
                if wide:
                    pw = psA.tile([P, 640], dtf, tag="psA")
                    ps = pw[:, :512]
                else:
                    ps = psB.tile([P, 512], dtf, tag="psB")
                emit_proj_seg(ot_sb, b, nt, oc, ps, 0, evac_act)
                emit_proj_seg(ot_sb, b, nt, oc, ps, 1, evac_act)

            # ---- phase 0: input DMAs in first-needed order ----
            x0 = load_x(0)
            for (g0, g1) in [(0, 256), (256, 512), (512, 1024), (1024, 1536),
                             (1536, 2048)]:
                for ct in range(CT):
                    eng = nc.gpsimd if ct % 2 == 0 else nc.sync
                    eng.dma_start(
                        out=wq_sb[:, ct, g0:g1],
                        in_=wq[ct * P:(ct + 1) * P, g0:g1],
                    )
                if g0 == 0:
                    nc.sync.dma_start(out=bqk_sb[:], in_=bqk[:])
                    nc.gpsimd.dma_start(out=idn_sb[:], in_=idn[:, :])
            for ct in range(CT):
                nc.sync.dma_start(
                    out=wq_sb[:, ct, 2 * C:], in_=wq[ct * P:(ct + 1) * P, 2 * C:]
                )
            nc.sync.dma_start(
                out=bvb_sb[:], in_=bass.AP(tensor=bv, offset=0, ap=[[0, P], [1, C]])
            )
            for ct in range(CT):
                nc.sync.dma_start(out=wp_sb[:, ct], in_=wp[ct * P:(ct + 1) * P, :])
            nc.sync.dma_start(
                out=bpb_sb[:], in_=bass.AP(tensor=bpr, offset=0, ap=[[0, P], [1, C]])
            )

            # ---- phase 1: QKV(b0) + V(b0), dense ----
            qk0 = qkpool.tile([P, 16, N], dtb, tag="qk")
            for ot in range(16):
                emit_qk_tile(x0, qk0, ot, wide=True)
            v0 = alloc_v(0)
            for nt in range(NT):
                for oc in range(2):
                    emit_v_chunk(x0, v0, nt, oc)

            # ---- phase 2 ----
            x1 = load_x(1)
            qk1 = qkpool.tile([P, 16, N], dtb, tag="qk")
            v1 = alloc_v(1)
            otn0 = onpool.tile([P, NT, H * 64], dtb, tag="otn")
            ott0 = opool.tile([P, CT, N], dtb, tag="ot")

            for ot in range(16):
                fq.extend(qk_fill_quanta(x1, qk1, ot))
            for nt in range(NT):
                fq.extend(v_fill_quanta(x1, v1, nt, 0))
            per = [2, 2, 2, 2, 2, 2, 2, 2]
            for hp in range(H // 2):
                emit_pair(qk0, v0, otn0, hp, st_pumps=(1, 2, 3))
                pump(per[hp])
                # transpose of the previous pair's columns: psB slot + ACT
                # evac land ahead of the next pair's exps
                if hp >= 1:
                    emit_transpose(otn0, ott0, hp - 1)
            while fq:
                pump(1)
            emit_transpose(otn0, ott0, 7)

            # ---- phase 3 ----
            otn1 = onpool.tile([P, NT, H * 64], dtb, tag="otn")
            ott1 = opool.tile([P, CT, N], dtb, tag="ot")
            for nt in range(NT):
                fq.extend(v_fill_quanta(x1, v1, nt, 1))
            for nt in range(NT):
                for oc in range(2):
                    fq.extend(proj_fill_quanta(ott0, 0, nt, oc))
            per = [1, 1, 1, 1, 1, 1, 1, 1]
            for hp in range(H // 2):
                emit_pair(qk1, v1, otn1, hp)
                pump(per[hp])
                if hp >= 1:
                    emit_transpose(otn1, ott1, hp - 1)
            while fq:
                pump(1)
            emit_transpose(otn1, ott1, 7)

            # ---- phase 4 ----
            chunks4 = [(nt, oc) for nt in range(NT) for oc in range(2)]
            cells4 = [None] * len(chunks4)

            def p4_seg0(i):
                nt, oc = chunks4[i]
                pw = psA.tile([P, 640], dtf, tag="psA", name="p4w")
                cells4[i] = pw[:, :512]
                emit_proj_seg(ott1, 1, nt, oc, cells4[i], 0)

            p4_seg0(0)
            p4_seg0(1)
            for i in range(len(chunks4)):
                if i + 2 < len(chunks4):
                    p4_seg0(i + 2)
                nt, oc = chunks4[i]
                emit_proj_seg(ott1, 1, nt, oc, cells4[i], 1)
    nc.compile()
    return nc


def kernel(x, w_qkv, b_qkv, w_proj, b_proj):
    global LAST_RESULT
    _ensure_ntff_hook()
    from concourse.bass_utils import run_bass_kernel_spmd

    bf16 = ml_dtypes.bfloat16
    x = np.asarray(x, dtype=np.float32)
    w_qkv = np.asarray(w_qkv, dtype=np.float32)
    b_qkv = np.asarray(b_qkv, dtype=np.float32)
    w_proj = np.asarray(w_proj, dtype=np.float32)
    b_proj = np.asarray(b_proj, dtype=np.float32)

    xT = np.ascontiguousarray(np.transpose(x, (0, 2, 1))).astype(bf16)  # [B, C, N]
    wqkvT = np.ascontiguousarray(w_qkv.T).astype(bf16)  # [C, 3C]
    wprojT = np.ascontiguousarray(w_proj.T).astype(bf16)  # [C, C]
    bqk = np.ascontiguousarray(b_qkv[:2 * C].reshape(16, P).T).astype(np.float32)
    bv = np.ascontiguousarray(b_qkv[2 * C:]).astype(bf16)
    bpr = np.ascontiguousarray(b_proj).astype(bf16)
    idn = np.eye(P, dtype=bf16)

    in_maps = []
    for i in range(NCORES):
        in_maps.append(
            {
                "xt": np.ascontiguousarray(xT[i * BPC:(i + 1) * BPC]),
                "wqkvT": wqkvT,
                "wprojT": wprojT,
                "bqk": bqk,
                "bv": bv,
                "bproj": bpr,
                "idn": idn,
            }
        )

    if "nc" not in _CACHE:
        _CACHE["nc"] = _build_nc()
    nc = _CACHE["nc"]

    res = run_bass_kernel_spmd(nc, in_maps, core_ids=list(range(NCORES)))
    LAST_RESULT = res
    out = np.concatenate([r["y"] for r in res.results], axis=0)
    return np.ascontiguousarray(out.astype(np.float32))


if __name__ == "__main__":
    rng = np.random.default_rng(0)
    x = rng.standard_normal((B, N, C), dtype=np.float32)
    w_qkv = rng.standard_normal((3 * C, C), dtype=np.float32) * C ** -0.5
    b_qkv = rng.standard_normal(3 * C).astype(np.float32) * 0.02
    w_proj = rng.standard_normal((C, C), dtype=np.float32) * C ** -0.5
    b_proj = rng.standard_normal(C).astype(np.float32) * 0.02
    out = kernel(x=x, w_qkv=w_qkv, b_qkv=b_qkv, w_proj=w_proj, b_proj=b_proj)
    print(out.shape, out.dtype)


# revision 18
# speedup vs baseline: 1.0154x; 1.0154x over previous
"""Multi-head attention block (B=16, N=577, C=1024, H=16) on 8 Trainium2 NeuronCores.

Sharding: data-parallel over batch — 2 batch elements per core, no collectives.

Device dataflow per batch element (fully "transposed" so no on-device transposes):
  inputs staged host-side: xT = x^T  [C,N] bf16, wqkvT = w_qkv^T [C,3C] bf16,
  wprojT = w_proj^T [C,C] bf16.
  qT,kT [o,n] <- (wqkvT tile).T @ xT      (o on partitions: per-head [64, n])
  V     [n,o] <- (xT tile).T @ wqkvT      (n on partitions: per-head [m, 64])
  S^T   [m,n] <- (kT_h tile [d,m]).T @ qT_h [d,n]          (d=64 contraction)
  P^T = exp(0.125 * S^T)                  (softmax numerator; max-subtraction skipped:
                                           scaled scores are ~N(0,1), |s|<~10, exp safe)
  OT'[0:64,n] = sum_m V_h[m,d] P^T[m,n];  OT'[64,n] = sum_m P^T[m,n]
      (one matmul: lhsT = [V_h | ones] [m, 65] — sumexp comes free as row 64)
  OT = OT'[0:64] * (1/OT'[64])            (softmax denominator)
  y[n,o] = (OT tile [c,n]).T @ wprojT + b_proj
"""

import os
import sys

import numpy as np

if "/opt/trn_rl_repo" not in sys.path:
    sys.path.insert(0, "/opt/trn_rl_repo")

import ml_dtypes

B, N, C = 16, 577, 1024
H, D = 16, 64
P = 128
CT = C // P  # 8 contraction tiles
NT = 5  # n(row) tiles of 128: 4*128 + 65
NTS = [128, 128, 128, 128, 65]
NCH = [(0, 512), (512, 65)]  # free-dim chunks of 577 (psum bank = 512 fp32)
NCORES = 8
BPC = B // NCORES  # batches per core

_CACHE = {}
LAST_RESULT = None


def _ensure_ntff_hook():
    try:
        from antenv import axon_hooks  # noqa: F401

        return
    except ImportError:
        pass
    import contextlib
    import ctypes
    import types

    import antenv

    so_path = "/opt/axon/libaxon_pjrt.so"
    mod = types.ModuleType("antenv.axon_hooks")
    _state = {"hook": None, "set": False}

    def _make_hook():
        if not os.path.exists(so_path):
            return None
        lib = ctypes.CDLL(so_path)
        if not hasattr(lib, "axon_start_nrt_profile"):
            return None
        lib.axon_start_nrt_profile.argtypes = [
            ctypes.POINTER(ctypes.c_int64),
            ctypes.c_size_t,
        ]
        lib.axon_start_nrt_profile.restype = ctypes.c_int64
        lib.axon_stop_nrt_profile.argtypes = [ctypes.c_char_p]
        lib.axon_stop_nrt_profile.restype = ctypes.c_int64

        @contextlib.contextmanager
        def _hook(output_dir, device_ids):
            import jax

            jax.devices()
            if device_ids:
                ids = (ctypes.c_int64 * len(device_ids))(*device_ids)
                rc = lib.axon_start_nrt_profile(ids, len(device_ids))
            else:
                rc = lib.axon_start_nrt_profile(None, 0)
            if rc != 0:
                raise RuntimeError(f"axon_start_nrt_profile rc={rc}")
            try:
                yield
            finally:
                n = lib.axon_stop_nrt_profile(str(output_dir).encode())
                print(f"ntff profile: {n} file(s) written to {output_dir}", file=sys.stderr)

        return _hook

    def set_axon_ntff_profile_hook(h):
        _state["hook"] = h
        _state["set"] = True

    def get_axon_ntff_profile_hook():
        if not _state["set"]:
            set_axon_ntff_profile_hook(_make_hook())
        return _state["hook"]

    mod.set_axon_ntff_profile_hook = set_axon_ntff_profile_hook
    mod.get_axon_ntff_profile_hook = get_axon_ntff_profile_hook
    sys.modules["antenv.axon_hooks"] = mod
    antenv.axon_hooks = mod


def _build_nc():
    import concourse.bass as bass
    import concourse.tile as tile
    from concourse import bacc, mybir

    dtb = mybir.dt.bfloat16
    dtf = mybir.dt.float32
    Exp = mybir.ActivationFunctionType.Exp

    nc = bacc.Bacc(None, target_bir_lowering=False)

    xt = nc.dram_tensor("xt", [BPC, C, N], dtb, kind="ExternalInput")
    wq = nc.dram_tensor("wqkvT", [C, 3 * C], dtb, kind="ExternalInput")
    wp = nc.dram_tensor("wprojT", [C, C], dtb, kind="ExternalInput")
    bqk = nc.dram_tensor("bqk", [P, 16], dtf, kind="ExternalInput")
    bv = nc.dram_tensor("bv", [C], dtb, kind="ExternalInput")
    bpr = nc.dram_tensor("bproj", [C], dtb, kind="ExternalInput")
    idn = nc.dram_tensor("idn", [P, P], dtb, kind="ExternalInput")
    y = nc.dram_tensor("y", [BPC, N, C], dtb, kind="ExternalOutput")

    from contextlib import ExitStack

    with tile.TileContext(nc) as tc:
        with ExitStack() as ctx:
            consts = ctx.enter_context(tc.tile_pool(name="consts", bufs=1))
            wpool = ctx.enter_context(tc.tile_pool(name="weights", bufs=1))
            xpool = ctx.enter_context(tc.tile_pool(name="xin", bufs=2))
            qkpool = ctx.enter_context(tc.tile_pool(name="qk", bufs=2))
            vpool = ctx.enter_context(tc.tile_pool(name="vv", bufs=2))
            epool = ctx.enter_context(tc.tile_pool(name="est", bufs=4))
            opool = ctx.enter_context(tc.tile_pool(name="ot", bufs=2))
            rpool = ctx.enter_context(tc.tile_pool(name="rp", bufs=4))
            onpool = ctx.enter_context(tc.tile_pool(name="on", bufs=2))
            outpool = ctx.enter_context(tc.tile_pool(name="outs", bufs=2))
            psA = ctx.enter_context(tc.tile_pool(name="psA", bufs=3, space="PSUM"))
            psB = ctx.enter_context(tc.tile_pool(name="psB", bufs=2, space="PSUM"))

            wq_sb = wpool.tile([P, CT, 3 * C], dtb, tag="wq")
            wp_sb = wpool.tile([P, CT, C], dtb, tag="wp")
            bqk_sb = consts.tile([P, 16], dtf, tag="bqk")
            ones1 = consts.tile([1, P], dtb, tag="ones1")
            nc.vector.memset(ones1[:], 1.0)
            wdum = consts.tile([1, 512], dtb, tag="wdum")
            nc.vector.memset(wdum[:], 1.0)
            pwarm = psB.tile([P, 512], dtf, tag="psB", name="pwarm")
            for i in range(7):
                nc.tensor.matmul(
                    pwarm[:], lhsT=ones1[0:1, :], rhs=wdum[0:1, :],
                    start=(i == 0), stop=(i == 6),
                )
            idn_sb = consts.tile([P, P], dtb, tag="idn")
            bvb_sb = consts.tile([P, C], dtb, tag="bvb")
            bpb_sb = consts.tile([P, C], dtb, tag="bpb")

            def load_x(b):
                x_sb = xpool.tile([P, CT, N], dtb, tag="x")
                xb = xt[b].rearrange("(ct p) n -> p ct n", p=P)
                for ct in range(CT):
                    eng = nc.sync if ct % 2 == 0 else nc.gpsimd
                    eng.dma_start(out=x_sb[:, ct], in_=xb[:, ct])
                return x_sb

            def emit_qk_tile(x_sb, qk_sb, ot, wide):
                if wide:
                    ps = psA.tile([P, 640], dtf, tag="psA")
                    for (c0, cw) in NCH:
                        for ct in range(CT):
                            nc.tensor.matmul(
                                ps[:, c0:c0 + cw],
                                lhsT=wq_sb[:, ct, ot * P:(ot + 1) * P],
                                rhs=x_sb[:, ct, c0:c0 + cw],
                                start=(ct == 0),
                                stop=(ct == CT - 1),
                            )
                    nc.vector.tensor_scalar_add(
                        out=qk_sb[:, ot, :],
                        in0=ps[:, :N],
                        scalar1=bqk_sb[:, ot:ot + 1],
                    )
                    return
                for (c0, cw) in NCH:
                    ps = psB.tile([P, 512], dtf, tag="psB")
                    for ct in range(CT):
                        nc.tensor.matmul(
                            ps[:, :cw],
                            lhsT=wq_sb[:, ct, ot * P:(ot + 1) * P],
                            rhs=x_sb[:, ct, c0:c0 + cw],
                            start=(ct == 0),
                            stop=(ct == CT - 1),
                        )
                    nc.vector.tensor_scalar_add(
                        out=qk_sb[:, ot, c0:c0 + cw],
                        in0=ps[:, :cw],
                        scalar1=bqk_sb[:, ot:ot + 1],
                    )

            def emit_v_chunk(x_sb, v_sb, nt, oc):
                nh = NTS[nt]
                ps = psB.tile([P, 512], dtf, tag="psB")
                for ct in range(CT):
                    nc.tensor.matmul(
                        ps[:nh],
                        lhsT=x_sb[:, ct, nt * P:nt * P + nh],
                        rhs=wq_sb[:, ct, 2 * C + oc * 512:2 * C + (oc + 1) * 512],
                        start=(ct == 0),
                        stop=(ct == CT - 1),
                    )
                vv = v_sb[:nh, nt, oc * 8 * 65:(oc + 1) * 8 * 65].rearrange(
                    "p (h c) -> p h c", c=65
                )
                nc.vector.tensor_add(
                    out=vv[:, :, 0:64],
                    in0=ps[:nh].rearrange("p (h c) -> p h c", c=64),
                    in1=bvb_sb[:nh, oc * 512:(oc + 1) * 512].rearrange(
                        "p (h c) -> p h c", c=64
                    ),
                )

            # ---- filler quanta: FIFO-pumped inside pairs so the PE keeps
            # streaming while ACT drains exp PSUM tiles. Strict FIFO keeps at
            # most one unit "open", so psB slot lifetimes never interleave.
            from collections import deque
            fq = deque()

            def pump(n=1):
                for _ in range(n):
                    if fq:
                        fq.popleft()()

            def qk_fill_quanta(x_sb, qk_sb, ot):
                box = {}

                def qa(ot=ot):
                    box["ps"] = psB.tile([P, 512], dtf, tag="psB", name="qkf")
                    for ct in range(4):
                        nc.tensor.matmul(
                            box["ps"][:, :512],
                            lhsT=wq_sb[:, ct, ot * P:(ot + 1) * P],
                            rhs=x_sb[:, ct, 0:512],
                            start=(ct == 0),
                            stop=False,
                        )

                def qb(ot=ot):
                    ps = box["ps"]
                    for ct in range(4, 8):
                        nc.tensor.matmul(
                            ps[:, :512],
                            lhsT=wq_sb[:, ct, ot * P:(ot + 1) * P],
                            rhs=x_sb[:, ct, 0:512],
                            start=False,
                            stop=(ct == 7),
                        )
                    nc.vector.tensor_scalar_add(
                        out=qk_sb[:, ot, 0:512],
                        in0=ps[:, :512],
                        scalar1=bqk_sb[:, ot:ot + 1],
                    )

                def qc(ot=ot):
                    ps2 = psB.tile([P, 512], dtf, tag="psB", name="qkf2")
                    for ct in range(CT):
                        nc.tensor.matmul(
                            ps2[:, :65],
                            lhsT=wq_sb[:, ct, ot * P:(ot + 1) * P],
                            rhs=x_sb[:, ct, 512:577],
                            start=(ct == 0),
                            stop=(ct == CT - 1),
                        )
                    nc.vector.tensor_scalar_add(
                        out=qk_sb[:, ot, 512:577],
                        in0=ps2[:, :65],
                        scalar1=bqk_sb[:, ot:ot + 1],
                    )

                return [qa, qb, qc]

            def v_fill_quanta(x_sb, v_sb, nt, oc):
                box = {}
                nh = NTS[nt]

                def seg(s):
                    ps = box["ps"]
                    for ct in range(s * 4, s * 4 + 4):
                        nc.tensor.matmul(
                            ps[:nh],
                            lhsT=x_sb[:, ct, nt * P:nt * P + nh],
                            rhs=wq_sb[:, ct,
                                      2 * C + oc * 512:2 * C + (oc + 1) * 512],
                            start=(ct == 0),
                            stop=(ct == CT - 1),
                        )

                def va():
                    box["ps"] = psB.tile([P, 512], dtf, tag="psB", name="vf")
                    seg(0)

                def vb():
                    seg(1)
                    vv = v_sb[:nh, nt, oc * 8 * 65:(oc + 1) * 8 * 65].rearrange(
                        "p (h c) -> p h c", c=65
                    )
                    nc.vector.tensor_add(
                        out=vv[:, :, 0:64],
                        in0=box["ps"][:nh].rearrange("p (h c) -> p h c", c=64),
                        in1=bvb_sb[:nh, oc * 512:(oc + 1) * 512].rearrange(
                            "p (h c) -> p h c", c=64
                        ),
                    )

                return [va, vb]

            def proj_fill_quanta(ot_sb, b, nt, oc):
                box = {}

                def pa():
                    box["ps"] = psB.tile([P, 512], dtf, tag="psB", name="pjf")
                    emit_proj_seg(ot_sb, b, nt, oc, box["ps"], 0)

                def pb():
                    emit_proj_seg(ot_sb, b, nt, oc, box["ps"], 1)

                return [pa, pb]

            def emit_transpose(otn_sb, ott_sb, ct):
                """OT [n, c-slice of pair ct] -> OT^T [c-slice, n] via 5 PE
                transposes into one bf16 psum bank, single ACT evac."""
                pst = psB.tile([P, 640], dtb, tag="psB", name="tp")
                for nb in range(NT):
                    nh2 = NTS[nb]
                    nc.tensor.transpose(
                        out=pst[:, nb * P:nb * P + nh2],
                        in_=otn_sb[:nh2, nb, ct * P:(ct + 1) * P],
                        identity=idn_sb[:nh2, :nh2],
                    )
                nc.scalar.copy(out=ott_sb[:, ct, :], in_=pst[:, :N])

            def alloc_v(b):
                v_sb = vpool.tile([P, NT, H * 65], dtb, tag="v")
                v4 = v_sb[:].rearrange("p nt (h c) -> p nt h c", c=65)
                nc.vector.memset(v4[:, :, :, 64], 1.0)
                return v_sb

            def emit_pair(qk_sb, v_sb, ot_sb, hp):
                ob = hp
                estA = epool.tile([P, NT, N], dtb, tag="est")
                estB = epool.tile([P, NT, N], dtb, tag="est")
                for mt in range(NT):
                    mh = NTS[mt]
                    psa = psA.tile([P, 640], dtf, tag="psA")
                    psb = psA.tile([P, 640], dtf, tag="psA")
                    for (c0, cw) in NCH:
                        nc.tensor.matmul(
                            psa[:mh, c0:c0 + cw],
                            lhsT=qk_sb[0:64, 8 + ob, mt * P:mt * P + mh],
                            rhs=qk_sb[0:64, ob, c0:c0 + cw],
                        )
                        nc.tensor.matmul(
                            psb[:mh, c0:c0 + cw],
                            lhsT=qk_sb[64:128, 8 + ob, mt * P:mt * P + mh],
                            rhs=qk_sb[64:128, ob, c0:c0 + cw],
                        )
                    nc.scalar.activation(
                        out=estA[:mh, mt, :], in_=psa[:mh, :N], func=Exp, scale=0.125
                    )
                    nc.scalar.activation(
                        out=estB[:mh, mt, :], in_=psb[:mh, :N], func=Exp, scale=0.125
                    )
                    if mt in (1, 3):
                        pump(1)
                # PV per head: one 2-bank psA tile holds all 5 n-block
                # chunks at 65-col offsets (bank 0), so the PE streams 25
                # back-to-back 65-col matmuls per head with no evac gating.
                recs = rpool.tile([P, 10], dtf, tag="rec")
                for hi, est in ((0, estA), (1, estB)):
                    h = 2 * hp + hi
                    pvt = psA.tile([P, 640], dtf, tag="psA", name="pv")
                    for nb in range(NT):
                        nh2 = NTS[nb]
                        for mt in range(NT):
                            mh = NTS[mt]
                            nc.tensor.matmul(
                                pvt[:nh2, nb * 65:nb * 65 + 65],
                                lhsT=est[:mh, mt, nb * P:nb * P + nh2],
                                rhs=v_sb[:mh, mt, h * 65:h * 65 + 65],
                                start=(mt == 0),
                                stop=(mt == NT - 1),
                            )
                    pv = pvt[:, :]
                    rc = recs[:, :]
                    on = ot_sb[:, :, :]
                    pstr = pv.ap[0][0]
                    # batched softmax denominators: the 5 sumexp columns
                    # (stride 65) in one reciprocal
                    nc.vector.reciprocal_approx_fast(
                        out=recs[:, hi * 5:hi * 5 + 5],
                        in_=bass.AP(tensor=pv.tensor, offset=pv.offset + 64,
                                    ap=[[pstr, P], [65, NT]]),
                    )
                    # normalize+evac: nb0-3 in one 0-stride-broadcast mul
                    nc.vector.tensor_mul(
                        out=bass.AP(tensor=on.tensor,
                                    offset=on.offset + h * 64,
                                    ap=[[on.ap[0][0], P], [H * 64, 4], [1, 64]]),
                        in0=bass.AP(tensor=pv.tensor, offset=pv.offset,
                                    ap=[[pstr, P], [65, 4], [1, 64]]),
                        in1=bass.AP(tensor=rc.tensor,
                                    offset=rc.offset + hi * 5,
                                    ap=[[rc.ap[0][0], P], [1, 4], [0, 64]]),
                    )
                    nc.vector.tensor_mul(
                        out=bass.AP(tensor=on.tensor,
                                    offset=on.offset + 4 * H * 64 + h * 64,
                                    ap=[[on.ap[0][0], 65], [1, 64]]),
                        in0=bass.AP(tensor=pv.tensor, offset=pv.offset + 260,
                                    ap=[[pstr, 65], [1, 64]]),
                        in1=bass.AP(tensor=rc.tensor,
                                    offset=rc.offset + hi * 5 + 4,
                                    ap=[[rc.ap[0][0], 65], [0, 64]]),
                    )
                    if hi == 0:
                        pump(1)

            def emit_proj_seg(ot_sb, b, nt, oc, ps, seg, evac_act=False):
                nh = NTS[nt]
                for ct in range(seg * 4, seg * 4 + 4):
                    nc.tensor.matmul(
                        ps[:nh],
                        lhsT=ot_sb[:, ct, nt * P:nt * P + nh],
                        rhs=wp_sb[:, ct, oc * 512:(oc + 1) * 512],
                        start=(ct == 0),
                        stop=(ct == CT - 1 and not evac_act),
                    )
                if seg == 1:
                    outt = outpool.tile([P, 512], dtb, tag="out")
                    if evac_act:
                        nc.tensor.matmul(
                            ps[:nh],
                            lhsT=ones1[0:1, :nh],
                            rhs=bpb_sb[0:1, oc * 512:(oc + 1) * 512],
                            start=False,
                            stop=True,
                        )
                        nc.scalar.copy(out=outt[:nh], in_=ps[:nh])
                    else:
                        nc.vector.tensor_add(
                            out=outt[:nh],
                            in0=ps[:nh],
                            in1=bpb_sb[:nh, oc * 512:(oc + 1) * 512],
                        )
                    if b == 1:
                        engs = [nc.sync, nc.gpsimd, nc.scalar]
                        eng = engs[(2 * nt + oc) % 3]
                    else:
                        eng = nc.sync if (nt + oc) % 2 == 0 else nc.gpsimd
                    eng.dma_start(
                        out=y[b, nt * P:nt * P + nh, oc * 512:(oc + 1) * 512],
                        in_=outt[:nh],
                    )

            def emit_proj_chunk(ot_sb, b, nt, oc, wide=False, evac_act=False):
                if wide:
                    pw = psA.tile([P, 640], dtf, tag="psA")
                    ps = pw[:, :512]
                else:
                    ps = psB.tile([P, 512], dtf, tag="psB")
                emit_proj_seg(ot_sb, b, nt, oc, ps, 0, evac_act)
                emit_proj_seg(ot_sb, b, nt, oc, ps, 1, evac_act)

            # ---- phase 0: input DMAs in first-needed order ----
            x0 = load_x(0)
            for (g0, g1) in [(0, 256), (256, 512), (512, 1024), (1024, 1536),
                             (1536, 2048)]:
                for ct in range(CT):
                    eng = nc.gpsimd if ct % 2 == 0 else nc.sync
                    eng.dma_start(
                        out=wq_sb[:, ct, g0:g1],
                        in_=wq[ct * P:(ct + 1) * P, g0:g1],
                    )
                if g0 == 0:
                    nc.sync.dma_start(out=bqk_sb[:], in_=bqk[:])
                    nc.gpsimd.dma_start(out=idn_sb[:], in_=idn[:, :])
            for ct in range(CT):
                nc.sync.dma_start(
                    out=wq_sb[:, ct, 2 * C:], in_=wq[ct * P:(ct + 1) * P, 2 * C:]
                )
            nc.sync.dma_start(
                out=bvb_sb[:], in_=bass.AP(tensor=bv, offset=0, ap=[[0, P], [1, C]])
            )
            for ct in range(CT):
                nc.sync.dma_start(out=wp_sb[:, ct], in_=wp[ct * P:(ct + 1) * P, :])
            nc.sync.dma_start(
                out=bpb_sb[:], in_=bass.AP(tensor=bpr, offset=0, ap=[[0, P], [1, C]])
            )

            # ---- phase 1: QKV(b0) + V(b0), dense ----
            qk0 = qkpool.tile([P, 16, N], dtb, tag="qk")
            for ot in range(16):
                emit_qk_tile(x0, qk0, ot, wide=True)
            v0 = alloc_v(0)
            for nt in range(NT):
                for oc in range(2):
                    emit_v_chunk(x0, v0, nt, oc)

            # ---- phase 2 ----
            x1 = load_x(1)
            qk1 = qkpool.tile([P, 16, N], dtb, tag="qk")
            v1 = alloc_v(1)
            otn0 = onpool.tile([P, NT, H * 64], dtb, tag="otn")
            ott0 = opool.tile([P, CT, N], dtb, tag="ot")

            for ot in range(16):
                fq.extend(qk_fill_quanta(x1, qk1, ot))
            for nt in range(NT):
                fq.extend(v_fill_quanta(x1, v1, nt, 0))
            per = [3, 3, 3, 2, 2, 2, 2, 2]
            for hp in range(H // 2):
                emit_pair(qk0, v0, otn0, hp)
                pump(per[hp])
                # transpose of the previous pair's columns: psB slot + ACT
                # evac land ahead of the next pair's exps
                if hp >= 1:
                    emit_transpose(otn0, ott0, hp - 1)
            while fq:
                pump(1)
            emit_transpose(otn0, ott0, 7)

            # ---- phase 3 ----
            otn1 = onpool.tile([P, NT, H * 64], dtb, tag="otn")
            ott1 = opool.tile([P, CT, N], dtb, tag="ot")
            for nt in range(NT):
                fq.extend(v_fill_quanta(x1, v1, nt, 1))
            for nt in range(NT):
                for oc in range(2):
                    fq.extend(proj_fill_quanta(ott0, 0, nt, oc))
            per = [1, 1, 1, 1, 1, 1, 1, 1]
            for hp in range(H // 2):
                emit_pair(qk1, v1, otn1, hp)
                pump(per[hp])
                if hp >= 1:
                    emit_transpose(otn1, ott1, hp - 1)
            while fq:
                pump(1)
            emit_transpose(otn1, ott1, 7)

            # ---- phase 4 ----
            chunks4 = [(nt, oc) for nt in range(NT) for oc in range(2)]
            cells4 = [None] * len(chunks4)

            def p4_seg0(i):
                nt, oc = chunks4[i]
                pw = psA.tile([P, 640], dtf, tag="psA", name="p4w")
                cells4[i] = pw[:, :512]
                emit_proj_seg(ott1, 1, nt, oc, cells4[i], 0)

            p4_seg0(0)
            p4_seg0(1)
            for i in range(len(chunks4)):
                if i + 2 < len(chunks4):
                    p4_seg0(i + 2)
                nt, oc = chunks4[i]
                emit_proj_seg(ott1, 1, nt, oc, cells4[i], 1)
    nc.compile()
    return nc


def kernel(x, w_qkv, b_qkv, w_proj, b_proj):
    global LAST_RESULT
    _ensure_ntff_hook()
    from concourse.bass_utils import run_bass_kernel_spmd

    bf16 = ml_dtypes.bfloat16
    x = np.asarray(x, dtype=np.float32)
    w_qkv = np.asarray(w_qkv, dtype=np.float32)
    b_qkv = np.asarray(b_qkv, dtype=np.float32)
    w_proj = np.asarray(w_proj, dtype=np.float32)
    b_proj = np.asarray(b_proj, dtype=np.float32)

    xT = np.ascontiguousarray(np.transpose(x, (0, 2, 1))).astype(bf16)  # [B, C, N]
    wqkvT = np.ascontiguousarray(w_qkv.T).astype(bf16)  # [C, 3C]
    wprojT = np.ascontiguousarray(w_proj.T).astype(bf16)  # [C, C]
    bqk = np.ascontiguousarray(b_qkv[:2 * C].reshape(16, P).T).astype(np.float32)
    bv = np.ascontiguousarray(b_qkv[2 * C:]).astype(bf16)
    bpr = np.ascontiguousarray(b_proj).astype(bf16)
    idn = np.eye(P, dtype=bf16)

    in_maps = []
    for i in range(NCORES):
        in_maps.append(
            {
                "xt": np.ascontiguousarray(xT[i * BPC:(i + 1) * BPC]),
                "wqkvT": wqkvT,
                "wprojT": wprojT,
                "bqk": bqk,
                "bv": bv,
                "bproj": bpr,
                "idn": idn,
            }
        )

    if "nc" not in _CACHE:
        _CACHE["nc"] = _build_nc()
    nc = _CACHE["nc"]

    res = run_bass_kernel_spmd(nc, in_maps, core_ids=list(range(NCORES)))
    LAST_RESULT = res
    out = np.concatenate([r["y"] for r in res.results], axis=0)
    return np.ascontiguousarray(out.astype(np.float32))


if __name__ == "__main__":
    rng = np.random.default_rng(0)
    x = rng.standard_normal((B, N, C), dtype=np.float32)
    w_qkv = rng.standard_normal((3 * C, C), dtype=np.float32) * C ** -0.5
    b_qkv = rng.standard_normal(3 * C).astype(np.float32) * 0.02
    w_proj = rng.standard_normal((C, C), dtype=np.float32) * C ** -0.5
    b_proj = rng.standard_normal(C).astype(np.float32) * 0.02
    out = kernel(x=x, w_qkv=w_qkv, b_qkv=b_qkv, w_proj=w_proj, b_proj=b_proj)
    print(out.shape, out.dtype)
